# revision 1
# baseline (speedup 1.0000x reference)
"""MoE ConditionalFeedForward kernel for 8 trn2 NeuronCores.

Strategy: expert parallelism. E=8 experts == 8 cores, so core k owns expert k's
weights (w1[k], w3[k], w2[k]) and processes exactly the (token, slot) pairs
routed to expert k. Routing/gather/scatter run on host; the heavy compute
(3 x C x D x I MACs per core over 1.1 GB of weights) runs on device.

Device math per core (C = padded token capacity, D=2048, I=5632):
  phase 1: hT[i, c] = silu(sum_d w1T[d,i] xT[d,c]) * (sum_d w3T[d,i] xT[d,c])
           (PE matmuls with d on partitions; w1/w3 pre-transposed on host)
  phase 2: y[c, d]  = sum_i hT[i, c] * w2[i, d]
           (PE matmuls with i on partitions; w2 in natural layout)

All weights/activations stream as bf16 (1 PE cycle/row vs 4 for f32; half the
HBM traffic); PSUM accumulation is f32 and the output is f32.
"""

import numpy as np
import ml_dtypes

BF16 = ml_dtypes.bfloat16

# Problem dims (hardcoded per contract; kernel.py must be self-contained).
T, A, E, D, I = 1024, 2, 8, 2048, 5632
N_CORES = 8

_BUILD_CACHE = {}


def _pick_groups(ib):
    """Blocks-per-DMA for the phase-1 (w1/w3) and phase-2 (w2) weight streams."""
    g1 = 2 if ib % 2 == 0 else 1
    g2 = 4 if ib % 4 == 0 else (2 if ib % 2 == 0 else 1)
    return g1, g2


def _pick_npass(d):
    """Split phase 2's D dim into npass passes so the live yT PSUM accumulator
    tags ((d/npass)/128 of them) fit in 8 banks. Prefer double-buffered
    (bufs=2) tags so consecutive passes overlap, then the fewest passes."""
    for bufs in (2, 1):
        for npass in (1, 2, 4, 8, 16):
            ndc = d // npass // 128
            if d % npass == 0 and (d // npass) % 128 == 0 and ndc * bufs <= 8:
                return npass, bufs
    raise ValueError(f"no valid npass for d={d}")


def _build(cap, d=D, i_dim=I):
    """Build + compile the per-core Bass program for token capacity `cap`."""
    key = (cap, d, i_dim)
    if key in _BUILD_CACHE:
        return _BUILD_CACHE[key]

    import concourse.mybir as mybir
    import concourse.tile as tile
    from concourse import bacc

    dt = mybir.dt
    WDT = dt.bfloat16
    F32 = dt.float32

    db = d // 128          # d-chunks (contraction of phase 1)
    ib = i_dim // 128      # i-blocks (contraction of phase 2)
    g1, g2 = _pick_groups(ib)
    ng1, ng2 = ib // g1, ib // g2
    assert cap % 32 == 0 and cap <= 512
    npass, ps_bufs = _pick_npass(d)
    w = d // npass         # output columns per phase-2 pass
    nw = w // 512          # 512-col chunks per pass

    nc = bacc.Bacc("TRN2", target_bir_lowering=False, debug=False,
                   num_devices=N_CORES)

    xgt = nc.dram_tensor("xgt", [128, db * cap], WDT, kind="ExternalInput").ap()
    w1d = nc.dram_tensor("w1d", [ng1, 128, g1 * db * 128], WDT,
                         kind="ExternalInput").ap()
    w3d = nc.dram_tensor("w3d", [ng1, 128, g1 * db * 128], WDT,
                         kind="ExternalInput").ap()
    w2d = nc.dram_tensor("w2d", [npass, 128, ib * w], WDT,
                         kind="ExternalInput").ap()
    # output is y transposed ([D, cap]) so phase 2 can make w2's d-columns the
    # stationary M dim (divides exactly -> no M padding) and write the PSUM
    # [d_block, c] tiles out contiguously; the host untransposes for free.
    yt = nc.dram_tensor("yt", [d, cap], F32, kind="ExternalOutput").ap()

    with tile.TileContext(nc) as tc:
        with (
            tc.tile_pool(name="xpool", bufs=1) as xpool,
            tc.tile_pool(name="w1pool", bufs=3) as w1pool,
            tc.tile_pool(name="w3pool", bufs=3) as w3pool,
            tc.tile_pool(name="w2pool", bufs=3) as w2pool,
            tc.tile_pool(name="hpool", bufs=1) as hpool,
            tc.tile_pool(name="spool", bufs=2) as spool,
            tc.tile_pool(name="opool", bufs=4) as opool,
        ):
            xg = xpool.tile([128, db * cap], WDT)
            # chunked so the first matmuls don't wait on the whole transfer
            xq = max(1, db // 4) * cap
            for q0 in range(0, db * cap, xq):
                nc.sync.dma_start(xg[:, q0:q0 + xq], xgt[:, q0:q0 + xq])
            h = hpool.tile([128, ib * cap], WDT)

            # ---- phase 1: hT blocks ----
            with tc.tile_pool(name="psA", bufs=2, space="PSUM") as psA:
                for g in range(ng1):
                    wt1 = w1pool.tile([128, g1 * db * 128], WDT, tag="w1")
                    nc.sync.dma_start(wt1[:], w1d[g])
                    wt3 = w3pool.tile([128, g1 * db * 128], WDT, tag="w3")
                    nc.sync.dma_start(wt3[:], w3d[g])
                    for s in range(g1):
                        b = g * g1 + s
                        ps1 = psA.tile([128, cap], F32, tag="ps1")
                        ps3 = psA.tile([128, cap], F32, tag="ps3")
                        for do in range(db):
                            lo = (s * db + do) * 128
                            nc.tensor.matmul(
                                ps1[:], wt1[:, lo:lo + 128],
                                xg[:, do * cap:(do + 1) * cap],
                                start=(do == 0), stop=(do == db - 1))
                        for do in range(db):
                            lo = (s * db + do) * 128
                            nc.tensor.matmul(
                                ps3[:], wt3[:, lo:lo + 128],
                                xg[:, do * cap:(do + 1) * cap],
                                start=(do == 0), stop=(do == db - 1))
                        sig = spool.tile([128, cap], F32, tag="sig")
                        nc.scalar.activation(
                            sig[:], ps1[:],
                            mybir.ActivationFunctionType.Sigmoid)
                        m1 = spool.tile([128, cap], F32, tag="m1")
                        nc.vector.tensor_mul(m1[:], sig[:], ps3[:])
                        nc.vector.tensor_mul(
                            h[:, b * cap:(b + 1) * cap], m1[:], ps1[:])

            # ---- phase 2: yT[d, c] = sum_b w2[b, d].T @ hT[b, c] ----
            # stationary = w2 128-column d-blocks (M=128 exact), moving = hT
            # (N=cap). Output tiles are yT blocks, accumulated over all i.
            ndc = w // 128                      # 128-col d-blocks per pass
            # w2 groups: ~8 i-blocks per DMA (1 MB) for full HBM efficiency
            gsz = 8
            groups = [(b0, min(gsz, ib - b0)) for b0 in range(0, ib, gsz)]
            with tc.tile_pool(name="psB", bufs=ps_bufs, space="PSUM") as psB:
                for ph in range(npass):
                    po = {}
                    for dc in range(ndc):
                        po[dc] = psB.tile([128, cap], F32, tag=f"yT{dc}",
                                          name=f"po{dc}")
                    for b0, nb in groups:
                        wt2 = w2pool.tile([128, gsz * w], WDT, tag="w2")
                        nc.sync.dma_start(wt2[:, :nb * w],
                                          w2d[ph][:, b0 * w:(b0 + nb) * w])
                        for s in range(nb):
                            b = b0 + s
                            for dc in range(ndc):
                                lo = s * w + dc * 128
                                nc.tensor.matmul(
                                    po[dc][:],
                                    wt2[:, lo:lo + 128],
                                    h[:, b * cap:(b + 1) * cap],
                                    start=(b == 0), stop=(b == ib - 1))
                    for dc in range(ndc):
                        ot = opool.tile([128, cap], F32, tag="ot")
                        nc.vector.tensor_copy(ot[:], po[dc][:])
                        nc.scalar.dma_start(
                            yt[ph * w + dc * 128:ph * w + dc * 128 + 128, :],
                            ot[:])

    nc.compile()
    _BUILD_CACHE[key] = nc
    return nc


def _pack_w13(wk, d=D, i_dim=I):
    """Host-side relayout of a [I, D] w1/w3 matrix into the pre-transposed
    phase-1 device layout (see _build)."""
    db, ib = d // 128, i_dim // 128
    g1, _ = _pick_groups(ib)
    ng1 = ib // g1
    # [g, s, i_in, do, di] -> [g, di, s, do, i_in]
    return np.ascontiguousarray(
        wk.reshape(ng1, g1, 128, db, 128).transpose(0, 4, 1, 3, 2)
    ).reshape(ng1, 128, g1 * db * 128)


def _pack_w2(w2k, npass, d=D, i_dim=I):
    """[I, D] -> [ph, i_in, b*w + dcol]: per-pass flat block-major layout so
    phase 2 can DMA any run of i-blocks as one big contiguous transfer."""
    ib = i_dim // 128
    w = d // npass
    # [b, i_in, ph, dcol] -> [ph, i_in, b, dcol]
    return np.ascontiguousarray(
        w2k.reshape(ib, 128, npass, w).transpose(2, 1, 0, 3)
    ).reshape(npass, 128, ib * w)


def _prepare(inputs):
    """Host routing + packing. Returns (nc, in_maps, scatter_info)."""
    x = np.asarray(inputs["x"])
    idx = np.asarray(inputs["expert_indices"])
    w1 = np.asarray(inputs["w1"])
    w2 = np.asarray(inputs["w2"])
    w3 = np.asarray(inputs["w3"])

    t, a = idx.shape
    d, i_dim = x.shape[1], w1.shape[1]
    db = d // 128

    # ---- host routing (the "all-to-all") ----
    flat = idx.reshape(-1).astype(np.int64)
    order = np.argsort(flat, kind="stable")          # pair ids grouped by expert
    counts = np.bincount(flat, minlength=E)
    starts = np.concatenate([[0], np.cumsum(counts)])
    cap = max(128, int(-(-counts.max() // 32) * 32))  # round up to mult of 32
    assert cap <= 512, f"capacity {cap} > 512 unsupported"
    npass, _ = _pick_npass(d)

    nc = _build(cap, d, i_dim)

    x_bf = x.astype(BF16)
    in_maps = []
    for k in range(E):
        sel = order[starts[k]:starts[k + 1]] // a      # token ids for expert k
        xg = np.zeros((cap, d), BF16)
        xg[:len(sel)] = x_bf[sel]
        # [c, d] -> [di, do, c]
        xgt = np.ascontiguousarray(
            xg.T.reshape(db, 128, cap).transpose(1, 0, 2)
        ).reshape(128, db * cap)
        w1d_ = _pack_w13(w1[k].astype(BF16), d, i_dim)
        w3d_ = _pack_w13(w3[k].astype(BF16), d, i_dim)
        w2d_ = _pack_w2(w2[k].astype(BF16), npass, d, i_dim)
        in_maps.append({"xgt": xgt, "w1d": w1d_, "w3d": w3d_, "w2d": w2d_})

    return nc, in_maps, (t, a, d, order, counts, starts)


def _scatter(results, scatter_info):
    t, a, d, order, counts, starts = scatter_info
    out_flat = np.zeros((t * a, d), np.float32)
    for k in range(E):
        n_k = int(counts[k])
        if n_k:
            out_flat[order[starts[k]:starts[k] + n_k]] = \
                results[k]["yt"][:, :n_k].T
    return out_flat.reshape(t, a, d)


def kernel(**inputs):
    from concourse.bass_utils import run_bass_kernel_spmd

    nc, in_maps, scatter_info = _prepare(inputs)
    res = run_bass_kernel_spmd(nc, in_maps, core_ids=list(range(N_CORES)))
    return _scatter(res.results, scatter_info)



# revision 2
# speedup vs baseline: 1.0308x; 1.0308x over previous
"""MoE ConditionalFeedForward kernel for 8 trn2 NeuronCores.

Strategy: expert parallelism. E=8 experts == 8 cores, so core k owns expert k's
weights (w1[k], w3[k], w2[k]) and processes exactly the (token, slot) pairs
routed to expert k. Routing/gather/scatter run on host; the heavy compute
(3 x C x D x I MACs per core over 1.1 GB of weights) runs on device.

Device math per core (C = padded token capacity, D=2048, I=5632):
  phase 1: hT[i, c] = silu(sum_d w1T[d,i] xT[d,c]) * (sum_d w3T[d,i] xT[d,c])
           (PE matmuls with d on partitions; w1/w3 pre-transposed on host)
  phase 2: y[c, d]  = sum_i hT[i, c] * w2[i, d]
           (PE matmuls with i on partitions; w2 in natural layout)

All weights/activations stream as bf16 (1 PE cycle/row vs 4 for f32; half the
HBM traffic); PSUM accumulation is f32 and the output is f32.

The kernel is PE-bound (2112 matmuls x cap rows at 2.4 GHz), so the layout is
tuned to keep the PE array saturated from the first microseconds:
  - cap is rounded to a multiple of 2 (not 32): PE time scales with cap.
  - the opening DMAs are interleaved in small chunks (x block 0, first w1
    columns, ...) so the first matmul can fire after ~200 KB instead of ~2 MB.
  - phase 2 uses one pass-persistent w2 tile (double buffered) filled by
    chunked DMAs, so each pass's weights prefetch a full pass ahead and the
    PSUM accumulators drain staggered, off the critical path.
"""

import numpy as np
import ml_dtypes

BF16 = ml_dtypes.bfloat16

# Problem dims (hardcoded per contract; kernel.py must be self-contained).
T, A, E, D, I = 1024, 2, 8, 2048, 5632
N_CORES = 8

G1 = 2        # i-blocks per w1/w3 DMA group (1 MB transfers)
NPASS = 4     # phase-2 passes over D (4 live PSUM accumulators, double buffered)
CHUNK_IB = 4  # i-blocks per w2 chunk DMA (0.5 MB transfers)

_BUILD_CACHE = {}


def _build(cap, d=D, i_dim=I):
    """Build + compile the per-core Bass program for token capacity `cap`."""
    key = (cap, d, i_dim)
    if key in _BUILD_CACHE:
        return _BUILD_CACHE[key]

    import concourse.mybir as mybir
    import concourse.tile as tile
    from concourse import bacc

    dt = mybir.dt
    WDT = dt.bfloat16
    F32 = dt.float32

    db = d // 128          # d-chunks (contraction of phase 1)
    ib = i_dim // 128      # i-blocks (contraction of phase 2)
    ng1 = ib // G1
    w = d // NPASS         # output columns per phase-2 pass
    ndc = w // 128         # 128-col d-blocks per pass
    assert cap % 2 == 0 and cap <= 512
    gcols = G1 * db * 128  # columns per w1/w3 group tile
    scols = db * 128       # columns per single i-block (s) within a group

    nc = bacc.Bacc("TRN2", target_bir_lowering=False, debug=False,
                   num_devices=N_CORES)

    xgt = nc.dram_tensor("xgt", [128, db * cap], WDT, kind="ExternalInput").ap()
    w1d = nc.dram_tensor("w1d", [ng1, 128, gcols], WDT,
                         kind="ExternalInput").ap()
    w3d = nc.dram_tensor("w3d", [ng1, 128, gcols], WDT,
                         kind="ExternalInput").ap()
    w2d = nc.dram_tensor("w2d", [NPASS, 128, ib * w], WDT,
                         kind="ExternalInput").ap()
    # output is y transposed ([D, cap]) so phase 2 can make w2's d-columns the
    # stationary M dim (divides exactly -> no M padding) and write the PSUM
    # [d_block, c] tiles out contiguously; the host untransposes for free.
    yt = nc.dram_tensor("yt", [d, cap], F32, kind="ExternalOutput").ap()

    with tile.TileContext(nc) as tc:
        with (
            tc.tile_pool(name="xpool", bufs=1) as xpool,
            tc.tile_pool(name="w1pool", bufs=3) as w1pool,
            tc.tile_pool(name="w3pool", bufs=3) as w3pool,
            tc.tile_pool(name="w2pool", bufs=2) as w2pool,
            tc.tile_pool(name="hpool", bufs=1) as hpool,
            tc.tile_pool(name="spool", bufs=2) as spool,
            tc.tile_pool(name="opool", bufs=4) as opool,
        ):
            xg = xpool.tile([128, db * cap], WDT)
            h = hpool.tile([128, ib * cap], WDT)

            # ---- phase 1: hT blocks ----
            with tc.tile_pool(name="psA", bufs=2, space="PSUM") as psA:
                for g in range(ng1):
                    wt1 = w1pool.tile([128, gcols], WDT, tag="w1")
                    wt3 = w3pool.tile([128, gcols], WDT, tag="w3")
                    if g == 0:
                        # Opening ramp: interleave x and the first w1 columns
                        # in small chunks so the first matmul fires after
                        # ~200 KB of DMA instead of the full x + w1 group.
                        q = 512  # 4 d-blocks of w1 columns per chunk
                        nc.sync.dma_start(xg[:, 0:cap], xgt[:, 0:cap])
                        nc.sync.dma_start(wt1[:, 0:q], w1d[0][:, 0:q])
                        nc.sync.dma_start(xg[:, cap:4 * cap],
                                          xgt[:, cap:4 * cap])
                        nc.sync.dma_start(wt1[:, q:2 * q], w1d[0][:, q:2 * q])
                        nc.sync.dma_start(xg[:, 4 * cap:10 * cap],
                                          xgt[:, 4 * cap:10 * cap])
                        nc.sync.dma_start(wt1[:, 2 * q:3 * q],
                                          w1d[0][:, 2 * q:3 * q])
                        nc.sync.dma_start(xg[:, 10 * cap:db * cap],
                                          xgt[:, 10 * cap:db * cap])
                        nc.sync.dma_start(wt1[:, 3 * q:scols],
                                          w1d[0][:, 3 * q:scols])
                        # w3 i-block 0, then the second i-block of each mat
                        nc.sync.dma_start(wt3[:, 0:scols], w3d[0][:, 0:scols])
                        nc.sync.dma_start(wt1[:, scols:gcols],
                                          w1d[0][:, scols:gcols])
                        nc.sync.dma_start(wt3[:, scols:gcols],
                                          w3d[0][:, scols:gcols])
                    else:
                        nc.sync.dma_start(wt1[:], w1d[g])
                        nc.sync.dma_start(wt3[:], w3d[g])
                    for s in range(G1):
                        b = g * G1 + s
                        ps1 = psA.tile([128, cap], F32, tag="ps1")
                        ps3 = psA.tile([128, cap], F32, tag="ps3")
                        for do in range(db):
                            lo = (s * db + do) * 128
                            nc.tensor.matmul(
                                ps1[:], wt1[:, lo:lo + 128],
                                xg[:, do * cap:(do + 1) * cap],
                                start=(do == 0), stop=(do == db - 1))
                        for do in range(db):
                            lo = (s * db + do) * 128
                            nc.tensor.matmul(
                                ps3[:], wt3[:, lo:lo + 128],
                                xg[:, do * cap:(do + 1) * cap],
                                start=(do == 0), stop=(do == db - 1))
                        sil = spool.tile([128, cap], F32, tag="sil")
                        nc.scalar.activation(
                            sil[:], ps1[:], mybir.ActivationFunctionType.Silu)
                        nc.vector.tensor_mul(
                            h[:, b * cap:(b + 1) * cap], sil[:], ps3[:])

            # ---- phase 2: yT[d, c] = sum_b w2[b, d].T @ hT[b, c] ----
            # stationary = w2 128-column d-blocks (M=128 exact), moving = hT
            # (N=cap). Each pass owns one persistent, double-buffered w2 tile
            # streamed in CHUNK_IB-block DMAs (pass p+1 prefetches during
            # pass p). The last ndc i-blocks of each accumulator run dc-major
            # so the PSUM drains stagger and hide behind the next dc's tail.
            nchunk = ib // CHUNK_IB
            with tc.tile_pool(name="psB", bufs=2, space="PSUM") as psB:
                for ph in range(NPASS):
                    wt2 = w2pool.tile([128, ib * w], WDT, tag="w2")
                    for c in range(nchunk):
                        c0 = c * CHUNK_IB * w
                        c1 = min(ib, (c + 1) * CHUNK_IB) * w
                        nc.sync.dma_start(wt2[:, c0:c1], w2d[ph][:, c0:c1])
                    po = {}
                    for dc in range(ndc):
                        po[dc] = psB.tile([128, cap], F32, tag=f"yT{dc}",
                                          name=f"po{dc}")
                    for b in range(ib - ndc):
                        for dc in range(ndc):
                            lo = b * w + dc * 128
                            nc.tensor.matmul(
                                po[dc][:], wt2[:, lo:lo + 128],
                                h[:, b * cap:(b + 1) * cap],
                                start=(b == 0), stop=False)
                    for dc in range(ndc):
                        for b in range(ib - ndc, ib):
                            lo = b * w + dc * 128
                            nc.tensor.matmul(
                                po[dc][:], wt2[:, lo:lo + 128],
                                h[:, b * cap:(b + 1) * cap],
                                start=False, stop=(b == ib - 1))
                        ot = opool.tile([128, cap], F32, tag="ot")
                        nc.vector.tensor_copy(ot[:], po[dc][:])
                        nc.scalar.dma_start(
                            yt[ph * w + dc * 128:ph * w + dc * 128 + 128, :],
                            ot[:])

    nc.compile()
    _BUILD_CACHE[key] = nc
    return nc


def _pack_w13(wk, d=D, i_dim=I):
    """Host-side relayout of a [I, D] w1/w3 matrix into the pre-transposed
    phase-1 device layout (see _build)."""
    db, ib = d // 128, i_dim // 128
    ng1 = ib // G1
    # [g, s, i_in, do, di] -> [g, di, s, do, i_in]
    return np.ascontiguousarray(
        wk.reshape(ng1, G1, 128, db, 128).transpose(0, 4, 1, 3, 2)
    ).reshape(ng1, 128, G1 * db * 128)


def _pack_w2(w2k, d=D, i_dim=I):
    """[I, D] -> [ph, i_in, b*w + dcol]: per-pass flat block-major layout so
    phase 2 can DMA any run of i-blocks as one big contiguous transfer."""
    ib = i_dim // 128
    w = d // NPASS
    # [b, i_in, ph, dcol] -> [ph, i_in, b, dcol]
    return np.ascontiguousarray(
        w2k.reshape(ib, 128, NPASS, w).transpose(2, 1, 0, 3)
    ).reshape(NPASS, 128, ib * w)


def _prepare(inputs):
    """Host routing + packing. Returns (nc, in_maps, scatter_info)."""
    x = np.asarray(inputs["x"])
    idx = np.asarray(inputs["expert_indices"])
    w1 = np.asarray(inputs["w1"])
    w2 = np.asarray(inputs["w2"])
    w3 = np.asarray(inputs["w3"])

    t, a = idx.shape
    d, i_dim = x.shape[1], w1.shape[1]
    db = d // 128

    # ---- host routing (the "all-to-all") ----
    flat = idx.reshape(-1).astype(np.int64)
    order = np.argsort(flat, kind="stable")          # pair ids grouped by expert
    counts = np.bincount(flat, minlength=E)
    starts = np.concatenate([[0], np.cumsum(counts)])
    cap = max(16, int(-(-counts.max() // 2) * 2))    # round up to mult of 2
    assert cap <= 512, f"capacity {cap} > 512 unsupported"

    nc = _build(cap, d, i_dim)

    x_bf = x.astype(BF16)
    in_maps = []
    for k in range(E):
        sel = order[starts[k]:starts[k + 1]] // a      # token ids for expert k
        xg = np.zeros((cap, d), BF16)
        xg[:len(sel)] = x_bf[sel]
        # [c, d] -> [di, do, c]
        xgt = np.ascontiguousarray(
            xg.T.reshape(db, 128, cap).transpose(1, 0, 2)
        ).reshape(128, db * cap)
        w1d_ = _pack_w13(w1[k].astype(BF16), d, i_dim)
        w3d_ = _pack_w13(w3[k].astype(BF16), d, i_dim)
        w2d_ = _pack_w2(w2[k].astype(BF16), d, i_dim)
        in_maps.append({"xgt": xgt, "w1d": w1d_, "w3d": w3d_, "w2d": w2d_})

    return nc, in_maps, (t, a, d, order, counts, starts)


def _scatter(results, scatter_info):
    t, a, d, order, counts, starts = scatter_info
    out_flat = np.zeros((t * a, d), np.float32)
    for k in range(E):
        n_k = int(counts[k])
        if n_k:
            out_flat[order[starts[k]:starts[k] + n_k]] = \
                results[k]["yt"][:, :n_k].T
    return out_flat.reshape(t, a, d)


def kernel(**inputs):
    from concourse.bass_utils import run_bass_kernel_spmd

    nc, in_maps, scatter_info = _prepare(inputs)
    res = run_bass_kernel_spmd(nc, in_maps, core_ids=list(range(N_CORES)))
    return _scatter(res.results, scatter_info)
